# revision 1
# baseline (speedup 1.0000x reference)
"""Trainium2 Bass kernel for a 4-layer GraphConv stack (GNN message passing).

Strategy (8 NeuronCores, SPMD, 5 NEFF dispatches):
  - Host relabels nodes (in-degree sort, deal round-robin to cores, then
    within-core (deg, lower-window-deg) sort) and bins edges by
    destination into padded per-128-node-block round-robin slot streams
    (int16, pre-split by source window since dma_gather indices are
    signed 16-bit; pads point at a dead always-zero row).
  - Dispatch 0 computes both degree norms on device (counting non-pad
    slots of int32 incidence tables for the graph and its transpose,
    then reciprocal/sqrt/mask) plus the first feature table shard
    h1 = z * norm_src.
  - Dispatches 1..4 run one GraphConv layer each: row-gather of the
    replicated feature table with dma_gather (one SWDGE descriptor per
    edge; the table is a pure ExternalInput - the custom gather crashes
    on device-written or scratchpad vars, HW-verified), pairwise tree
    adds on VectorE, norm_dst scale, PE transpose, matmul with W,
    bias+ReLU on ScalarE, PE transpose back, norm_src scale for the
    next layer's gather. The host concatenates the 8 shard outputs into
    the next layer's replicated table (pure index routing).

Host python does only index marshaling and array routing; all
arithmetic on tensor data happens on the NeuronCores.
"""

import math

import numpy as np

import concourse.bacc as bacc
import concourse.bass as bass
import concourse.mybir as mybir
import concourse.tile as tile
from concourse.bass_utils import run_bass_kernel_spmd

P = 128
NC = 8
DIMS = [32, 32, 64, 128, 128]
ES = [32, 32, 64, 128]      # gathered row width per layer (floats)
TW = [64, 64, 64, 128]      # table row stride per layer (floats, 256B mult)
F32 = mybir.dt.float32
I32 = mybir.dt.int32
I16 = mybir.dt.int16


class Cfg:
    def __init__(self, n_nodes):
        assert n_nodes % NC == 0
        self.N = n_nodes
        self.NREAL = n_nodes // NC
        # at least one dead (always-zero) row per core: the pad target
        self.BPC = math.ceil((self.NREAL + 1) / P)
        self.NS = self.BPC * P
        self.NT = NC * self.NS
        self.SPLIT = (NC // 2) * self.NS
        assert self.SPLIT <= 32767 and self.NT - self.SPLIT <= 32767
        self.ZR = self.NT


# ---------------------------------------------------------------- host prep

def _wrap16(stream):
    n = len(stream)
    assert n % 128 == 0
    t = np.empty((16, n // 16), np.int16)
    t[np.arange(n) % 16, np.arange(n) // 16] = stream
    return np.tile(t, (8, 1))


def build_structures(cfg, src, dst):
    N, NS, BPC, ZR = cfg.N, cfg.NS, cfg.BPC, cfg.ZR
    NREAL, SPLIT = cfg.NREAL, cfg.SPLIT
    src = np.asarray(src, np.int64)
    dst = np.asarray(dst, np.int64)

    in_deg = np.bincount(dst, minlength=N)
    out_deg = np.bincount(src, minlength=N)

    order = np.argsort(-in_deg, kind="stable")
    core_of = np.empty(N, np.int64)
    core_of[order] = np.arange(N) % NC
    srcA = core_of[src] < NC // 2
    degA = np.bincount(dst[srcA], minlength=N)

    new_of_old = np.empty(N, np.int64)
    for c in range(NC):
        nodes = np.where(core_of == c)[0]
        o = np.lexsort((-degA[nodes], -in_deg[nodes]))
        new_of_old[nodes[o]] = c * NS + np.arange(len(nodes))

    src_n = new_of_old[src]
    dst_n = new_of_old[dst]
    degB = in_deg - degA

    KA = np.zeros(BPC, np.int64)
    KB = np.zeros(BPC, np.int64)
    K = np.zeros(BPC, np.int64)
    K2 = np.zeros(BPC, np.int64)
    blk_of_old = (new_of_old % NS) // P
    for b in range(BPC):
        m = blk_of_old == b
        if m.any():
            KA[b] = degA[m].max()
            KB[b] = degB[m].max()
            K[b] = in_deg[m].max()
            K2[b] = out_deg[m].max()
    KA, KB = np.maximum(KA, 1), np.maximum(KB, 1)
    K, K2 = np.maximum(K, 1), np.maximum(K2, 1)
    CSA = np.concatenate([[0], np.cumsum(KA)]).astype(np.int64)
    CSB = np.concatenate([[0], np.cumsum(KB)]).astype(np.int64)
    CS = np.concatenate([[0], np.cumsum(K)]).astype(np.int64)
    CS2 = np.concatenate([[0], np.cumsum(K2)]).astype(np.int64)
    SA, SB = int(CSA[-1]), int(CSB[-1])
    S, S2 = int(CS[-1]), int(CS2[-1])

    def fill_stream(loc_dst, val, K_, CS_, S_, pad):
        stream = np.full(S_ * P, pad, np.int64)
        o = np.argsort(loc_dst, kind="stable")
        kk, vv = loc_dst[o], val[o]
        starts = np.searchsorted(kk, np.arange(NS))
        rank = np.arange(len(kk)) - starts[kk]
        b = kk // P
        pp = kk % P
        assert (rank < K_[b]).all()
        stream[(CS_[b] + rank) * P + pp] = vv
        return stream.astype(np.int16)

    def make_tab(key, val, S_, CS_, K_):
        o = np.argsort(key, kind="stable")
        kk, vv = key[o], val[o]
        starts = np.searchsorted(kk, np.arange(NS))
        rank = np.arange(len(kk)) - starts[kk]
        b = kk // P
        pp = kk % P
        assert (rank < K_[b]).all()
        tab = np.full((P, S_), ZR, np.int32)
        tab[pp, CS_[b] + rank] = vv
        return tab

    idx16_tabs, slot_tabs, cnt_tabs = [], [], []
    for c in range(NC):
        own = (dst_n >= c * NS) & (dst_n < (c + 1) * NS)
        eA = own & srcA
        eB = own & ~srcA
        sa = fill_stream(dst_n[eA] - c * NS, src_n[eA], KA, CSA, SA, NREAL)
        sb = fill_stream(dst_n[eB] - c * NS, src_n[eB] - SPLIT, KB, CSB, SB, NREAL)
        idx16_tabs.append(np.concatenate([_wrap16(sa), _wrap16(sb)], axis=1))
        slot_tabs.append(make_tab(dst_n[own] - c * NS, src_n[own], S, CS, K))
        own_s = (src_n >= c * NS) & (src_n < (c + 1) * NS)
        cnt_tabs.append(make_tab(src_n[own_s] - c * NS, dst_n[own_s], S2, CS2, K2))

    return dict(new_of_old=new_of_old, KA=KA, KB=KB, CSA=CSA, CSB=CSB,
                SA=SA, SB=SB, K=K, CS=CS, S=S, K2=K2, CS2=CS2, S2=S2,
                idx16_tabs=idx16_tabs, slot_tabs=slot_tabs, cnt_tabs=cnt_tabs)


# ------------------------------------------------------------- bass helpers

def _dma_gather_raw(nc, out_ap, in_ap, idxs_ap, num_idxs, elem_size, elem_step):
    """nc.gpsimd.dma_gather minus the 256B elem_size assert (128B elems are
    fine for the non-transpose path, HW-verified; the row stride must be a
    256B multiple)."""
    gp = nc.gpsimd
    stride_bytes = elem_step * 4
    assert stride_bytes % 256 == 0 and stride_bytes // 256 < 256
    assert num_idxs % 128 == 0
    _in_ap = gp.lower_ap_dma(in_ap, for_custom_bir_dma=True)
    _idxs_ap = gp.lower_ap(idxs_ap)
    _out_ap = gp.lower_ap(out_ap)
    return gp.add_instruction(
        mybir.InstDMAGatherAnt(
            name=gp.bass.get_next_instruction_name(),
            ins=[*_in_ap, _idxs_ap, gp.lower_val_access(gp.to_reg(num_idxs))],
            outs=[_out_ap],
            transpose=False,
            num_idxs=num_idxs,
            elem_size=elem_size,
            stride_bytes_256=stride_bytes // 256,
            gen_mode=0,
            single_packet=True,
            queue_num=0,
            sbuf_tokens_per_rank=0,
            sbuf_free_dim_per_rank=0,
            sbuf_free_dim_pad_per_rank=0,
            sbuf_byte_offset=0,
        )
    )


def _tree_reduce(nc, g, w, d, acc, first):
    while w > 1:
        h = (w + 1) // 2
        lo = w - h
        nc.vector.tensor_add(
            out=g[:, : lo * d], in0=g[:, : lo * d], in1=g[:, h * d : w * d]
        )
        w = h
    if first:
        nc.vector.tensor_copy(out=acc[:], in_=g[:, :d])
    else:
        nc.vector.tensor_add(out=acc[:], in0=acc[:], in1=g[:, :d])


def _count_degrees(nc, pool, tab_sb, CS_, BPC, zr, deg_out):
    S_ = int(CS_[-1])
    ind = pool.tile([P, S_], F32, tag="ind")
    nc.vector.tensor_scalar(
        out=ind[:], in0=tab_sb[:], scalar1=float(zr), scalar2=None,
        op0=mybir.AluOpType.is_lt,
    )
    for b in range(BPC):
        nc.vector.tensor_reduce(
            out=deg_out[:, b : b + 1],
            in_=ind[:, int(CS_[b]) : int(CS_[b + 1])],
            axis=mybir.AxisListType.X,
            op=mybir.AluOpType.add,
        )


def _norm_from_deg(nc, pool, deg, norm, BPC):
    m = pool.tile([P, BPC], F32, tag="nmask")
    safe = pool.tile([P, BPC], F32, tag="nsafe")
    nc.vector.tensor_scalar(
        out=m[:], in0=deg[:], scalar1=0.0, scalar2=None,
        op0=mybir.AluOpType.is_gt,
    )
    nc.vector.tensor_scalar(
        out=safe[:], in0=deg[:], scalar1=1.0, scalar2=None,
        op0=mybir.AluOpType.max,
    )
    nc.vector.reciprocal(out=safe[:], in_=safe[:])
    nc.scalar.sqrt(out=safe[:], in_=safe[:])
    nc.vector.tensor_mul(out=norm[:], in0=safe[:], in1=m[:])


def _groups(cfg, Kh, capcols):
    out = []
    b = 0
    while b < cfg.BPC:
        e = b + 1
        tot = Kh[b]
        while e < cfg.BPC and tot + Kh[e] <= capcols:
            tot += Kh[e]
            e += 1
        out.append((b, e))
        b = e
    return out


def _new_nc():
    return bacc.Bacc(
        "TRN2", target_bir_lowering=False, debug=False, num_devices=NC
    )


def build_norm_program(cfg, st):
    """Dispatch 0: degree norms + h1 shard = z * norm_src (padded)."""
    NS, BPC, ZR = cfg.NS, cfg.BPC, cfg.ZR
    CS, CS2, S, S2 = st["CS"], st["CS2"], st["S"], st["S2"]
    nc = _new_nc()
    z_in = nc.dram_tensor("z_shard", [NS, DIMS[0]], F32, kind="ExternalInput")
    slot_in = nc.dram_tensor("slots", [P, S], I32, kind="ExternalInput")
    cnt_in = nc.dram_tensor("cnts", [P, S2], I32, kind="ExternalInput")
    nd_out = nc.dram_tensor("nd", [P, BPC], F32, kind="ExternalOutput")
    ns_out = nc.dram_tensor("ns", [P, BPC], F32, kind="ExternalOutput")
    h1_out = nc.dram_tensor("h1_shard", [NS, DIMS[0]], F32, kind="ExternalOutput")

    with tile.TileContext(nc) as tc:
        with tc.tile_pool(name="pro", bufs=1) as pro:
            norm_dst = pro.tile([P, BPC], F32, tag="ndst")
            norm_src = pro.tile([P, BPC], F32, tag="nsrc")
            slot_sb = pro.tile([P, S], I32, tag="slots")
            nc.sync.dma_start(out=slot_sb[:], in_=slot_in[:, :])
            deg = pro.tile([P, BPC], F32, tag="deg")
            _count_degrees(nc, pro, slot_sb, CS, BPC, ZR, deg)
            _norm_from_deg(nc, pro, deg, norm_dst, BPC)
            cnt_sb = pro.tile([P, S2], I32, tag="cnts")
            nc.sync.dma_start(out=cnt_sb[:], in_=cnt_in[:, :])
            deg2 = pro.tile([P, BPC], F32, tag="deg2")
            _count_degrees(nc, pro, cnt_sb, CS2, BPC, ZR, deg2)
            _norm_from_deg(nc, pro, deg2, norm_src, BPC)
            nc.sync.dma_start(out=nd_out[:, :], in_=norm_dst[:])
            nc.sync.dma_start(out=ns_out[:, :], in_=norm_src[:])

            zero32 = pro.tile([P, 32], F32, tag="zero32")
            nc.vector.memset(zero32[:], 0.0)
            with tc.tile_pool(name="zp", bufs=3) as zp:
                for b in range(BPC):
                    zt = zp.tile([P, DIMS[0]], F32, tag="z")
                    nc.sync.dma_start(out=zt[:], in_=z_in[b * P : (b + 1) * P, :])
                    nc.vector.tensor_mul(
                        out=zt[:], in0=zt[:],
                        in1=norm_src[:, b : b + 1].to_broadcast([P, DIMS[0]]),
                    )
                    nc.sync.dma_start(
                        out=h1_out[b * P : (b + 1) * P, :], in_=zt[:]
                    )
    nc.compile()
    return nc


def build_layer_program(cfg, st, l):
    """Dispatch l+1: one GraphConv layer. Per-column indirect row gathers
    (128 rows per op) from a pure-input feature table with a trailing
    zero row for pad slots."""
    NS, NT, BPC, ZR = cfg.NS, cfg.NT, cfg.BPC, cfg.ZR
    K, CS, S = st["K"], st["CS"], st["S"]
    d_in, d_out = DIMS[l], DIMS[l + 1]
    es = d_in
    last = l == 3

    nc = _new_nc()
    htab = nc.dram_tensor("htab", [NT + 1, es], F32, kind="ExternalInput")
    slot_in = nc.dram_tensor("slots", [P, S], I32, kind="ExternalInput")
    nd_in = nc.dram_tensor("nd", [P, BPC], F32, kind="ExternalInput")
    ns_in = nc.dram_tensor("ns", [P, BPC], F32, kind="ExternalInput")
    W_in = nc.dram_tensor("W", [d_in, d_out], F32, kind="ExternalInput")
    b_in = nc.dram_tensor("b", [d_out], F32, kind="ExternalInput")
    out_ext = nc.dram_tensor("out_shard", [NS, d_out], F32, kind="ExternalOutput")

    from concourse.masks import make_identity

    with tile.TileContext(nc) as tc:
        with tc.tile_pool(name="res", bufs=1) as res:
            slot_sb = res.tile([P, S], I32, tag="slots")
            nc.sync.dma_start(out=slot_sb[:], in_=slot_in[:, :])
            ident = res.tile([P, P], F32, tag="ident")
            make_identity(nc, ident[:])
            norm_dst = res.tile([P, BPC], F32, tag="ndst")
            nc.sync.dma_start(out=norm_dst[:], in_=nd_in[:, :])
            norm_src = res.tile([P, BPC], F32, tag="nsrc")
            nc.sync.dma_start(out=norm_src[:], in_=ns_in[:, :])
            W_sb = res.tile([d_in, d_out], F32, tag="W")
            nc.sync.dma_start(out=W_sb[:], in_=W_in[:, :])
            b_sb = res.tile([d_out, 1], F32, tag="b")
            nc.sync.dma_start(out=b_sb[:], in_=b_in[:, None])

            with (
                tc.tile_pool(name="g", bufs=8) as gp,
                tc.tile_pool(name="a", bufs=4) as ap,
                tc.tile_pool(name="ps", bufs=2, space="PSUM") as pp,
            ):
                for b in range(BPC):
                    acc = ap.tile([P, es], F32, tag="acc")
                    for k in range(int(K[b])):
                        g = gp.tile([P, es], F32, tag="g")
                        nc.gpsimd.indirect_dma_start(
                            out=g[:], out_offset=None, in_=htab[:, :],
                            in_offset=bass.IndirectOffsetOnAxis(
                                ap=slot_sb[:, int(CS[b]) + k : int(CS[b]) + k + 1],
                                axis=0,
                            ),
                        )
                        if k == 0:
                            nc.vector.tensor_copy(out=acc[:], in_=g[:])
                        else:
                            nc.vector.tensor_add(out=acc[:], in0=acc[:], in1=g[:])
                    nc.vector.tensor_mul(
                        out=acc[:], in0=acc[:],
                        in1=norm_dst[:, b : b + 1].to_broadcast([P, es]),
                    )
                    p1 = pp.tile([d_in, P], F32, tag="t1", space="PSUM")
                    nc.tensor.transpose(out=p1[:], in_=acc[:, :d_in], identity=ident[:])
                    accT = ap.tile([d_in, P], F32, tag="accT")
                    nc.scalar.copy(out=accT[:], in_=p1[:])
                    p2 = pp.tile([d_out, P], F32, tag="mm", space="PSUM")
                    nc.tensor.matmul(
                        out=p2[:], lhsT=W_sb[:], rhs=accT[:], start=True, stop=True
                    )
                    yT = ap.tile([d_out, P], F32, tag="yT")
                    nc.scalar.activation(
                        out=yT[:], in_=p2[:],
                        func=mybir.ActivationFunctionType.Relu,
                        bias=b_sb[:, :1],
                    )
                    p3 = pp.tile([P, d_out], F32, tag="t2", space="PSUM")
                    nc.tensor.transpose(
                        out=p3[:], in_=yT[:], identity=ident[:d_out, :d_out]
                    )
                    yb = ap.tile([P, d_out], F32, tag="yb")
                    if last:
                        nc.vector.tensor_copy(out=yb[:], in_=p3[:])
                    else:
                        nc.vector.tensor_mul(
                            out=yb[:], in0=p3[:],
                            in1=norm_src[:, b : b + 1].to_broadcast([P, d_out]),
                        )
                    nc.sync.dma_start(
                        out=out_ext[b * P : (b + 1) * P, :], in_=yb[:]
                    )
    nc.compile()
    return nc


# ------------------------------------------------------------------ driver

_prog_cache = {}
LAST_RESULTS = []


def _programs(cfg, st, key):
    if key not in _prog_cache:
        _prog_cache[key] = (
            build_norm_program(cfg, st),
            [build_layer_program(cfg, st, l) for l in range(4)],
        )
    return _prog_cache[key]


def kernel(z, src, dst, W1, b1, W2, b2, W3, b3, W4, b4, **extra):
    Ws = [np.ascontiguousarray(np.asarray(w, np.float32)) for w in (W1, W2, W3, W4)]
    bs = [np.ascontiguousarray(np.asarray(b, np.float32)) for b in (b1, b2, b3, b4)]
    z = np.ascontiguousarray(np.asarray(z, np.float32))
    cfg = Cfg(z.shape[0])
    st = build_structures(cfg, src, dst)
    key = (z.shape[0], st["S"], st["S2"], st["SA"], st["SB"],
           tuple(st["KA"]), tuple(st["KB"]))
    nc0, ncl = _programs(cfg, st, key)
    cores = list(range(NC))
    NS = cfg.NS

    z_all = np.zeros((cfg.NT, DIMS[0]), np.float32)
    z_all[st["new_of_old"]] = z

    in_maps = [
        {
            "z_shard": z_all[c * NS : (c + 1) * NS],
            "slots": st["slot_tabs"][c],
            "cnts": st["cnt_tabs"][c],
        }
        for c in range(NC)
    ]
    LAST_RESULTS.clear()
    _r = run_bass_kernel_spmd(nc0, in_maps, cores)
    LAST_RESULTS.append(_r)
    r0 = _r.results
    nds = [r["nd"] for r in r0]
    nss = [r["ns"] for r in r0]
    htab = np.concatenate([r["h1_shard"] for r in r0], axis=0)

    for l in range(4):
        htz = np.concatenate([htab, np.zeros((1, htab.shape[1]), np.float32)], axis=0)
        in_maps = [
            {
                "htab": htz,
                "slots": st["slot_tabs"][c],
                "nd": nds[c],
                "ns": nss[c],
                "W": Ws[l],
                "b": bs[l],
            }
            for c in range(NC)
        ]
        _r = run_bass_kernel_spmd(ncl[l], in_maps, cores)
        LAST_RESULTS.append(_r)
        rl = _r.results
        htab = np.concatenate([r["out_shard"] for r in rl], axis=0)

    return np.ascontiguousarray(htab[st["new_of_old"]])



# revision 4
# speedup vs baseline: 1.3389x; 1.3389x over previous
"""Trainium2 Bass kernel for a 4-layer GraphConv stack (GNN message passing).

Single fused NEFF dispatch on 8 NeuronCores (SPMD):
  - Host relabels nodes (in-degree sort, deal round-robin to cores, then
    within-core degree sort) and bins edges by destination into per-128-node-
    block slot-column streams, split into two source windows (A: cores 0-3,
    B: cores 4-7) so indices fit signed int16 for the SWDGE gather ucode.
    Pad slots point at a dead (always-zero) table row.
  - On device: degree norms are computed from int32 incidence tables
    (count non-pad slots, rsqrt, mask); h1 = z * norm_src is written to a
    bf16 shard bounce and AllGathered into the layer-1 feature table.
  - Each layer gathers source rows with batched InstDMAGatherAnt SWDGE
    gathers (<=1024 indices per instruction, round-robin over 4 SWDGE
    queues for parallel Q7 descriptor generation), tree-adds slot columns
    per dst block on VectorE, then PE-transposes, matmuls with W (bf16),
    and applies ReLU with both degree norms folded into the per-partition
    activation scale (valid since biases are zero and norms are >=0;
    a separate program variant handles nonzero bias via a ones-row matmul).
    Layer outputs land in a bf16 bounce, AllGathered into the next table.
  - Feature tables are [NT, 128] bf16 with rows on a 256B stride (SWDGE
    stride must be a 256B multiple); gathers read only the valid elem bytes.

Host python does only index marshaling and array routing; all arithmetic on
tensor data happens on the NeuronCores.
"""

import math

import numpy as np

import concourse.ap_utils as ap_utils
import concourse.bacc as bacc
import concourse.bass as bass
import concourse.mybir as mybir
import concourse.tile as tile
from concourse._compat import exact_div, round_up_to_multiple
from concourse.bass_utils import run_bass_kernel_spmd

P = 128
NC = 8
NQ = 4                       # SWDGE queues (ucode max)
MAXI = 1024                  # max idxs per gather instruction (HW-verified)
DIMS = [32, 32, 64, 128, 128]
TW = 128                     # table row stride in bf16 elems (256B)
F32 = mybir.dt.float32
BF16 = mybir.dt.bfloat16
I32 = mybir.dt.int32
I16 = mybir.dt.int16


class Cfg:
    def __init__(self, n_nodes):
        assert n_nodes % NC == 0
        self.N = n_nodes
        self.NREAL = n_nodes // NC
        # at least one dead (always-zero) row per core: the pad target
        self.BPC = math.ceil((self.NREAL + 1) / P)
        self.NS = self.BPC * P
        self.NT = NC * self.NS
        self.SPLIT = (NC // 2) * self.NS
        assert self.SPLIT <= 32767 and self.NT - self.SPLIT <= 32767
        self.PAD_A = self.NREAL            # core 0's dead row (window A)
        self.PAD_B = self.NREAL            # core 4's dead row - SPLIT (window B)


# ---------------------------------------------------------------- host prep

def _wrap16(stream):
    n = len(stream)
    assert n % 128 == 0
    t = np.empty((16, n // 16), np.int16)
    t[np.arange(n) % 16, np.arange(n) // 16] = stream
    return np.tile(t, (8, 1))


def build_structures(cfg, src, dst):
    N, NS, BPC = cfg.N, cfg.NS, cfg.BPC
    NREAL, SPLIT, NT = cfg.NREAL, cfg.SPLIT, cfg.NT
    src = np.asarray(src, np.int64)
    dst = np.asarray(dst, np.int64)

    in_deg = np.bincount(dst, minlength=N)
    out_deg = np.bincount(src, minlength=N)

    order = np.argsort(-in_deg, kind="stable")
    core_of = np.empty(N, np.int64)
    core_of[order] = np.arange(N) % NC
    srcA = core_of[src] < NC // 2
    degA = np.bincount(dst[srcA], minlength=N)

    new_of_old = np.empty(N, np.int64)
    for c in range(NC):
        nodes = np.where(core_of == c)[0]
        o = np.lexsort((-degA[nodes], -in_deg[nodes]))
        new_of_old[nodes[o]] = c * NS + np.arange(len(nodes))

    src_n = new_of_old[src]
    dst_n = new_of_old[dst]
    degB = in_deg - degA

    KA = np.zeros(BPC, np.int64)
    KB = np.zeros(BPC, np.int64)
    K = np.zeros(BPC, np.int64)
    K2 = np.zeros(BPC, np.int64)
    blk_of_old = (new_of_old % NS) // P
    for b in range(BPC):
        m = blk_of_old == b
        if m.any():
            KA[b] = degA[m].max()
            KB[b] = degB[m].max()
            K[b] = in_deg[m].max()
            K2[b] = out_deg[m].max()
    KA, KB = np.maximum(KA, 1), np.maximum(KB, 1)
    K, K2 = np.maximum(K, 1), np.maximum(K2, 1)
    CSA = np.concatenate([[0], np.cumsum(KA)]).astype(np.int64)
    CSB = np.concatenate([[0], np.cumsum(KB)]).astype(np.int64)
    CS = np.concatenate([[0], np.cumsum(K)]).astype(np.int64)
    CS2 = np.concatenate([[0], np.cumsum(K2)]).astype(np.int64)
    SA, SB = int(CSA[-1]), int(CSB[-1])
    S, S2 = int(CS[-1]), int(CS2[-1])

    def fill_stream(loc_dst, val, K_, CS_, S_, pad):
        stream = np.full(S_ * P, pad, np.int64)
        o = np.argsort(loc_dst, kind="stable")
        kk, vv = loc_dst[o], val[o]
        starts = np.searchsorted(kk, np.arange(NS))
        rank = np.arange(len(kk)) - starts[kk]
        b = kk // P
        pp = kk % P
        assert (rank < K_[b]).all()
        stream[(CS_[b] + rank) * P + pp] = vv
        return stream.astype(np.int16)

    def make_tab(key, val, S_, CS_, K_, pad):
        o = np.argsort(key, kind="stable")
        kk, vv = key[o], val[o]
        starts = np.searchsorted(kk, np.arange(NS))
        rank = np.arange(len(kk)) - starts[kk]
        b = kk // P
        pp = kk % P
        assert (rank < K_[b]).all()
        tab = np.full((P, S_), pad, np.int32)
        tab[pp, CS_[b] + rank] = vv
        return tab

    streamA_tabs, streamB_tabs, slot_tabs, cnt_tabs = [], [], [], []
    for c in range(NC):
        own = (dst_n >= c * NS) & (dst_n < (c + 1) * NS)
        eA = own & srcA
        eB = own & ~srcA
        sa = fill_stream(dst_n[eA] - c * NS, src_n[eA], KA, CSA, SA, cfg.PAD_A)
        sb = fill_stream(dst_n[eB] - c * NS, src_n[eB] - SPLIT, KB, CSB, SB,
                         cfg.PAD_B)
        streamA_tabs.append(_wrap16(sa))
        streamB_tabs.append(_wrap16(sb))
        slot_tabs.append(make_tab(dst_n[own] - c * NS, src_n[own], S, CS, K, NT))
        own_s = (src_n >= c * NS) & (src_n < (c + 1) * NS)
        cnt_tabs.append(make_tab(src_n[own_s] - c * NS, dst_n[own_s], S2, CS2,
                                 K2, NT))

    return dict(new_of_old=new_of_old, KA=KA, KB=KB, CSA=CSA, CSB=CSB,
                SA=SA, SB=SB, K=K, CS=CS, S=S, K2=K2, CS2=CS2, S2=S2,
                streamA_tabs=streamA_tabs, streamB_tabs=streamB_tabs,
                slot_tabs=slot_tabs, cnt_tabs=cnt_tabs)


# ------------------------------------------------------------- bass helpers

def _raw_gather(nc, out_ap, in_ap, idxs_ap, num_idxs, elem_size, elem_step,
                queue_num):
    """Official dma_gather lowering minus the 256B elem_size assert
    (64B/128B elems HW-verified on this runtime). in_ap is [rows, elem_size]
    with row stride elem_step."""
    gp = nc.gpsimd
    assert idxs_ap.dtype == mybir.dt.int16
    assert in_ap.dtype == out_ap.dtype
    assert ap_utils.ap_is_contiguous(out_ap.ap[1:])
    assert ap_utils.ap_is_contiguous(idxs_ap.ap[1:])
    assert in_ap.ap[-1][1] == out_ap.ap[-1][1] == elem_size
    assert out_ap.ap[0][1] * out_ap.ap[1][1] == round_up_to_multiple(num_idxs, 128)
    assert in_ap.ap[0][0] == elem_step
    stride_bytes = elem_step * mybir.dt.size(in_ap.dtype)
    stride_bytes_256 = exact_div(stride_bytes, 256)
    assert stride_bytes_256 < 256
    _in_ap = gp.lower_ap_dma(in_ap, for_custom_bir_dma=True)
    _idxs_ap = gp.lower_ap(idxs_ap)
    _out_ap = gp.lower_ap(out_ap)
    return gp.add_instruction(
        mybir.InstDMAGatherAnt(
            name=gp.bass.get_next_instruction_name(),
            ins=[*_in_ap, _idxs_ap, gp.lower_val_access(gp.to_reg(num_idxs))],
            outs=[_out_ap],
            transpose=False,
            num_idxs=num_idxs,
            elem_size=elem_size,
            stride_bytes_256=stride_bytes_256,
            gen_mode=0,
            single_packet=True,
            queue_num=queue_num,
            sbuf_tokens_per_rank=0,
            sbuf_free_dim_per_rank=0,
            sbuf_free_dim_pad_per_rank=0,
            sbuf_byte_offset=0,
        )
    )


def _count_degrees(nc, pool, tab_sb, CS_, BPC, zr, deg_out):
    S_ = int(CS_[-1])
    ind = pool.tile([P, S_], F32, tag="ind")
    nc.vector.tensor_scalar(
        out=ind[:], in0=tab_sb[:], scalar1=float(zr), scalar2=None,
        op0=mybir.AluOpType.is_lt,
    )
    for b in range(BPC):
        nc.vector.tensor_reduce(
            out=deg_out[:, b : b + 1],
            in_=ind[:, int(CS_[b]) : int(CS_[b + 1])],
            axis=mybir.AxisListType.X,
            op=mybir.AluOpType.add,
        )


def _norm_from_deg(nc, pool, deg, norm, BPC):
    m = pool.tile([P, BPC], F32, tag="nmask")
    safe = pool.tile([P, BPC], F32, tag="nsafe")
    nc.vector.tensor_scalar(
        out=m[:], in0=deg[:], scalar1=0.0, scalar2=None,
        op0=mybir.AluOpType.is_gt,
    )
    nc.vector.tensor_scalar(
        out=safe[:], in0=deg[:], scalar1=1.0, scalar2=None,
        op0=mybir.AluOpType.max,
    )
    nc.vector.reciprocal(out=safe[:], in_=safe[:])
    nc.scalar.sqrt(out=safe[:], in_=safe[:])
    nc.vector.tensor_mul(out=norm[:], in0=safe[:], in1=m[:])


def _tree(nc, region, w, es):
    """In-place pairwise tree-add of w columns of width es inside region."""
    while w > 1:
        h = (w + 1) // 2
        lo = w - h
        nc.vector.tensor_add(
            out=region[:, : lo * es], in0=region[:, : lo * es],
            in1=region[:, h * es : w * es],
        )
        w = h


def _groups(cfg, KA, KB, capcols):
    """Group consecutive blocks so each window's column total <= capcols."""
    out = []
    b = 0
    while b < cfg.BPC:
        e = b + 1
        ta, tb = KA[b], KB[b]
        while e < cfg.BPC and ta + KA[e] <= capcols and tb + KB[e] <= capcols:
            ta += KA[e]
            tb += KB[e]
            e += 1
        out.append((b, e))
        b = e
    return out


# ------------------------------------------------------------- the program

def build_program(cfg, st, has_bias):
    NS, NT, BPC, SPLIT = cfg.NS, cfg.NT, cfg.BPC, cfg.SPLIT
    KA, KB, CSA, CSB = st["KA"], st["KB"], st["CSA"], st["CSB"]
    SA, SB = st["SA"], st["SB"]
    CS, S, CS2, S2 = st["CS"], st["S"], st["CS2"], st["S2"]

    nc = bacc.Bacc("TRN2", target_bir_lowering=False, debug=False,
                   num_devices=NC, num_swdge_queues=NQ)

    z_in = nc.dram_tensor("z_shard", [NS, DIMS[0]], F32, kind="ExternalInput")
    sA_in = nc.dram_tensor("streamA", [128, SA * 8], I16, kind="ExternalInput")
    sB_in = nc.dram_tensor("streamB", [128, SB * 8], I16, kind="ExternalInput")
    slot_in = nc.dram_tensor("slots", [P, S], I32, kind="ExternalInput")
    cnt_in = nc.dram_tensor("cnts", [P, S2], I32, kind="ExternalInput")
    W_ins = [
        nc.dram_tensor(f"W{l+1}", [DIMS[l] + (1 if has_bias else 0), DIMS[l + 1]],
                       F32, kind="ExternalInput")
        for l in range(4)
    ]
    out_ext = nc.dram_tensor("out_shard", [NS, DIMS[4]], F32,
                             kind="ExternalOutput")

    from concourse.masks import make_identity

    qctr = [0]

    def next_q():
        q = qctr[0] % NQ
        qctr[0] += 1
        return q

    def gather_cols(res_tile, tab, es, idx_sb, c0, c1, col_off):
        """Gather stream columns [c0, c1) into res_tile at column offset."""
        cols = c1 - c0
        done = 0
        while done < cols:
            take = min(8, cols - done)
            ni = take * 128
            dst = res_tile[:, (col_off + done) * es : (col_off + done + take) * es]
            _raw_gather(
                nc, dst.rearrange("p (c d) -> p c d", d=es), tab,
                idx_sb[:, (c0 + done) * 8 : (c0 + done + take) * 8],
                ni, es, TW, next_q(),
            )
            done += take

    with tile.TileContext(nc) as tc:
        with tc.tile_pool(name="dram", bufs=1, space="DRAM") as dram:
            tables = [dram.tile([NT, TW], BF16, name=f"tab{l}") for l in range(4)]
            bounces = [dram.tile([NS, TW], BF16, name=f"bnc{l}") for l in range(4)]
            with tc.tile_pool(name="res", bufs=1) as res:
                # ---- persistent loads
                sA_sb = res.tile([128, SA * 8], I16, tag="sA")
                nc.sync.dma_start(out=sA_sb[:], in_=sA_in[:, :])
                sB_sb = res.tile([128, SB * 8], I16, tag="sB")
                nc.sync.dma_start(out=sB_sb[:], in_=sB_in[:, :])
                ident = res.tile([P, P], BF16, tag="ident")
                make_identity(nc, ident[:])
                W_sbs = []
                for l in range(4):
                    win = DIMS[l] + (1 if has_bias else 0)
                    wf = res.tile([win, DIMS[l + 1]], F32, tag=f"Wf{l}")
                    nc.sync.dma_start(out=wf[:], in_=W_ins[l][:, :])
                    wb = res.tile([win, DIMS[l + 1]], BF16, tag=f"Wb{l}")
                    nc.vector.tensor_copy(out=wb[:], in_=wf[:])
                    W_sbs.append(wb)

                # ---- degree norms
                norm_dst = res.tile([P, BPC], F32, tag="ndst")
                norm_src = res.tile([P, BPC], F32, tag="nsrc")
                norm_comb = res.tile([P, BPC], F32, tag="ncomb")
                with tc.tile_pool(name="deg", bufs=1) as dp:
                    slot_sb = dp.tile([P, S], I32, tag="slots")
                    nc.sync.dma_start(out=slot_sb[:], in_=slot_in[:, :])
                    deg = dp.tile([P, BPC], F32, tag="deg")
                    _count_degrees(nc, dp, slot_sb, CS, BPC, NT, deg)
                    _norm_from_deg(nc, dp, deg, norm_dst, BPC)
                    cnt_sb = dp.tile([P, S2], I32, tag="cnts")
                    nc.sync.dma_start(out=cnt_sb[:], in_=cnt_in[:, :])
                    deg2 = dp.tile([P, BPC], F32, tag="deg2")
                    _count_degrees(nc, dp, cnt_sb, CS2, BPC, NT, deg2)
                    _norm_from_deg(nc, dp, deg2, norm_src, BPC)
                    nc.vector.tensor_mul(
                        out=norm_comb[:], in0=norm_dst[:], in1=norm_src[:]
                    )

                # ---- h1 = z * norm_src -> bounce0 -> AllGather tab0
                with tc.tile_pool(name="zp", bufs=3) as zp:
                    for b in range(BPC):
                        zt = zp.tile([P, DIMS[0]], F32, tag="z")
                        nc.sync.dma_start(
                            out=zt[:], in_=z_in[b * P : (b + 1) * P, :]
                        )
                        zb = zp.tile([P, DIMS[0]], BF16, tag="zb")
                        nc.vector.tensor_mul(
                            out=zb[:], in0=zt[:],
                            in1=norm_src[:, b : b + 1].to_broadcast([P, DIMS[0]]),
                        )
                        nc.sync.dma_start(
                            out=bounces[0][b * P : (b + 1) * P, 0 : DIMS[0]],
                            in_=zb[:],
                        )
                nc.gpsimd.collective_compute(
                    "AllGather", mybir.AluOpType.bypass,
                    replica_groups=[list(range(NC))],
                    ins=[bounces[0].opt()], outs=[tables[0].opt()],
                )

                # ---- layers
                CAP = 64
                groups = _groups(cfg, KA, KB, CAP)
                for l in range(4):
                    es, d_out = DIMS[l], DIMS[l + 1]
                    last = l == 3
                    tabA = tables[l][:, 0:es]
                    tabB = tables[l][SPLIT:, 0:es]
                    with (
                        tc.tile_pool(name=f"g{l}", bufs=2) as gp,
                        tc.tile_pool(name=f"a{l}", bufs=4) as ap,
                        tc.tile_pool(name=f"ps{l}", bufs=4, space="PSUM") as pp,
                    ):
                        for (b0, b1) in groups:
                            a0, a1 = int(CSA[b0]), int(CSA[b1])
                            bb0, bb1 = int(CSB[b0]), int(CSB[b1])
                            gA = gp.tile([P, (a1 - a0) * es], BF16, tag="gA")
                            gB = gp.tile([P, (bb1 - bb0) * es], BF16, tag="gB")
                            gather_cols(gA, tabA, es, sA_sb, a0, a1, 0)
                            gather_cols(gB, tabB, es, sB_sb, bb0, bb1, 0)
                            for b in range(b0, b1):
                                ka, kb = int(KA[b]), int(KB[b])
                                oa = (int(CSA[b]) - a0) * es
                                ob = (int(CSB[b]) - bb0) * es
                                rA = gA[:, oa : oa + ka * es]
                                rB = gB[:, ob : ob + kb * es]
                                _tree(nc, rA, ka, es)
                                _tree(nc, rB, kb, es)
                                acc = ap.tile([P, es], BF16, tag="acc")
                                nc.vector.tensor_add(
                                    out=acc[:], in0=rA[:, :es], in1=rB[:, :es]
                                )
                                if has_bias:
                                    nc.vector.tensor_mul(
                                        out=acc[:], in0=acc[:],
                                        in1=norm_dst[:, b : b + 1]
                                        .to_broadcast([P, es]),
                                    )
                                p1 = pp.tile([es, P], BF16, tag="t1", space="PSUM")
                                nc.tensor.transpose(
                                    out=p1[:], in_=acc[:], identity=ident[:]
                                )
                                ein = es + (1 if has_bias else 0)
                                accT = ap.tile([ein, P], BF16, tag="accT")
                                nc.scalar.copy(out=accT[:es, :], in_=p1[:])
                                if has_bias:
                                    nc.vector.memset(accT[es : es + 1, :], 1.0)
                                p2 = pp.tile([P, d_out], F32, tag="mm",
                                             space="PSUM")
                                nc.tensor.matmul(
                                    out=p2[:], lhsT=accT[:], rhs=W_sbs[l][:],
                                    start=True, stop=True,
                                )
                                if last:
                                    yb = ap.tile([P, d_out], F32, tag="ybf")
                                    nc.scalar.activation(
                                        out=yb[:], in_=p2[:],
                                        func=mybir.ActivationFunctionType.Relu,
                                        scale=(1.0 if has_bias
                                               else norm_dst[:, b : b + 1]),
                                    )
                                    nc.sync.dma_start(
                                        out=out_ext[b * P : (b + 1) * P, :],
                                        in_=yb[:],
                                    )
                                else:
                                    yb = ap.tile([P, d_out], BF16, tag="yb")
                                    sc = norm_src if has_bias else norm_comb
                                    nc.scalar.activation(
                                        out=yb[:], in_=p2[:],
                                        func=mybir.ActivationFunctionType.Relu,
                                        scale=sc[:, b : b + 1],
                                    )
                                    nc.sync.dma_start(
                                        out=bounces[l + 1][
                                            b * P : (b + 1) * P, 0:d_out
                                        ],
                                        in_=yb[:],
                                    )
                    if not last:
                        nc.gpsimd.collective_compute(
                            "AllGather", mybir.AluOpType.bypass,
                            replica_groups=[list(range(NC))],
                            ins=[bounces[l + 1].opt()],
                            outs=[tables[l + 1].opt()],
                        )
    nc.compile()
    return nc


# ------------------------------------------------------------------ driver

_prog_cache = {}
LAST_RESULTS = []


def kernel(z, src, dst, W1, b1, W2, b2, W3, b3, W4, b4, **extra):
    Ws = [np.ascontiguousarray(np.asarray(w, np.float32)) for w in (W1, W2, W3, W4)]
    bs = [np.ascontiguousarray(np.asarray(b, np.float32)) for b in (b1, b2, b3, b4)]
    z = np.ascontiguousarray(np.asarray(z, np.float32))
    has_bias = any(np.any(b != 0) for b in bs)
    cfg = Cfg(z.shape[0])
    st = build_structures(cfg, src, dst)
    key = (z.shape[0], has_bias, st["SA"], st["SB"], st["S"], st["S2"],
           tuple(st["KA"]), tuple(st["KB"]))
    if key not in _prog_cache:
        _prog_cache[key] = build_program(cfg, st, has_bias)
    nc = _prog_cache[key]
    NS = cfg.NS

    z_all = np.zeros((cfg.NT, DIMS[0]), np.float32)
    z_all[st["new_of_old"]] = z

    if has_bias:
        W_full = [np.concatenate([w, b[None, :]], axis=0) for w, b in zip(Ws, bs)]
    else:
        W_full = Ws

    in_maps = [
        {
            "z_shard": z_all[c * NS : (c + 1) * NS],
            "streamA": st["streamA_tabs"][c],
            "streamB": st["streamB_tabs"][c],
            "slots": st["slot_tabs"][c],
            "cnts": st["cnt_tabs"][c],
            **{f"W{l+1}": W_full[l] for l in range(4)},
        }
        for c in range(NC)
    ]
    LAST_RESULTS.clear()
    _r = run_bass_kernel_spmd(nc, in_maps, list(range(NC)))
    LAST_RESULTS.append(_r)
    out_full = np.concatenate([r["out_shard"] for r in _r.results], axis=0)
    return np.ascontiguousarray(out_full[st["new_of_old"]])


# revision 6
# speedup vs baseline: 1.4468x; 1.0806x over previous
"""Trainium2 Bass kernel for a 4-layer GraphConv stack (GNN message passing).

Single fused NEFF dispatch on 8 NeuronCores (SPMD):
  - Host relabels nodes (in-degree sort, deal round-robin to cores, then
    within-core degree sort) and bins edges by destination into per-128-node-
    block slot-column streams, split into two source windows (A: cores 0-3,
    B: cores 4-7) so indices fit signed int16 for the SWDGE gather ucode.
    Pad slots point at a dead (always-zero) table row.
  - On device: degree norms are computed from int32 incidence tables
    (count non-pad slots, rsqrt, mask); h1 = z * norm_src is written to a
    bf16 shard bounce and AllGathered into the layer-1 feature table.
  - Each layer gathers source rows with batched InstDMAGatherAnt SWDGE
    gathers (<=1024 indices per instruction, round-robin over 4 SWDGE
    queues for parallel Q7 descriptor generation), tree-adds slot columns
    per dst block on VectorE, then PE-transposes, matmuls with W (bf16),
    and applies ReLU with both degree norms folded into the per-partition
    activation scale (valid since biases are zero and norms are >=0;
    a separate program variant handles nonzero bias via a ones-row matmul).
    Layer outputs land in a bf16 bounce, AllGathered into the next table.
  - Feature tables are [NT, 128] bf16 with rows on a 256B stride (SWDGE
    stride must be a 256B multiple); gathers read only the valid elem bytes.

Host python does only index marshaling and array routing; all arithmetic on
tensor data happens on the NeuronCores.
"""

import math

import numpy as np

import concourse.ap_utils as ap_utils
import concourse.bacc as bacc
import concourse.bass as bass
import concourse.mybir as mybir
import concourse.tile as tile
from concourse._compat import exact_div, round_up_to_multiple
from concourse.bass_utils import run_bass_kernel_spmd

P = 128
NC = 8
NQ = 4                       # SWDGE queues (ucode max)
MAXI = 1024                  # max idxs per gather instruction (HW-verified)
DIMS = [32, 32, 64, 128, 128]
TW = 128                     # table row stride in bf16 elems (256B)
F32 = mybir.dt.float32
BF16 = mybir.dt.bfloat16
I32 = mybir.dt.int32
I16 = mybir.dt.int16


class Cfg:
    def __init__(self, n_nodes):
        assert n_nodes % NC == 0
        self.N = n_nodes
        self.NREAL = n_nodes // NC
        # at least one dead (always-zero) row per core: the pad target
        self.BPC = math.ceil((self.NREAL + 1) / P)
        self.NS = self.BPC * P
        self.NT = NC * self.NS
        self.SPLIT = (NC // 2) * self.NS
        assert self.SPLIT <= 32767 and self.NT - self.SPLIT <= 32767
        self.PAD_A = self.NREAL            # core 0's dead row (window A)
        self.PAD_B = self.NREAL            # core 4's dead row - SPLIT (window B)


# ---------------------------------------------------------------- host prep

def _wrap16(stream):
    n = len(stream)
    assert n % 128 == 0
    t = np.empty((16, n // 16), np.int16)
    t[np.arange(n) % 16, np.arange(n) // 16] = stream
    return np.tile(t, (8, 1))


def build_structures(cfg, src, dst):
    N, NS, BPC = cfg.N, cfg.NS, cfg.BPC
    NREAL, SPLIT, NT = cfg.NREAL, cfg.SPLIT, cfg.NT
    src = np.asarray(src, np.int64)
    dst = np.asarray(dst, np.int64)

    in_deg = np.bincount(dst, minlength=N)
    out_deg = np.bincount(src, minlength=N)

    order = np.argsort(-in_deg, kind="stable")
    core_of = np.empty(N, np.int64)
    core_of[order] = np.arange(N) % NC
    srcA = core_of[src] < NC // 2
    degA = np.bincount(dst[srcA], minlength=N)

    new_of_old = np.empty(N, np.int64)
    for c in range(NC):
        nodes = np.where(core_of == c)[0]
        o = np.lexsort((-degA[nodes], -in_deg[nodes]))
        new_of_old[nodes[o]] = c * NS + np.arange(len(nodes))

    src_n = new_of_old[src]
    dst_n = new_of_old[dst]
    degB = in_deg - degA

    KA = np.zeros(BPC, np.int64)
    KB = np.zeros(BPC, np.int64)
    K = np.zeros(BPC, np.int64)
    K2 = np.zeros(BPC, np.int64)
    blk_of_old = (new_of_old % NS) // P
    for b in range(BPC):
        m = blk_of_old == b
        if m.any():
            KA[b] = degA[m].max()
            KB[b] = degB[m].max()
            K[b] = in_deg[m].max()
            K2[b] = out_deg[m].max()
    KA, KB = np.maximum(KA, 1), np.maximum(KB, 1)
    K, K2 = np.maximum(K, 1), np.maximum(K2, 1)
    CSA = np.concatenate([[0], np.cumsum(KA)]).astype(np.int64)
    CSB = np.concatenate([[0], np.cumsum(KB)]).astype(np.int64)
    CS = np.concatenate([[0], np.cumsum(K)]).astype(np.int64)
    CS2 = np.concatenate([[0], np.cumsum(K2)]).astype(np.int64)
    SA, SB = int(CSA[-1]), int(CSB[-1])
    S, S2 = int(CS[-1]), int(CS2[-1])

    def fill_stream(loc_dst, val, K_, CS_, S_, pad):
        stream = np.full(S_ * P, pad, np.int64)
        o = np.argsort(loc_dst, kind="stable")
        kk, vv = loc_dst[o], val[o]
        starts = np.searchsorted(kk, np.arange(NS))
        rank = np.arange(len(kk)) - starts[kk]
        b = kk // P
        pp = kk % P
        assert (rank < K_[b]).all()
        stream[(CS_[b] + rank) * P + pp] = vv
        return stream.astype(np.int16)

    def make_tab(key, val, S_, CS_, K_, pad):
        o = np.argsort(key, kind="stable")
        kk, vv = key[o], val[o]
        starts = np.searchsorted(kk, np.arange(NS))
        rank = np.arange(len(kk)) - starts[kk]
        b = kk // P
        pp = kk % P
        assert (rank < K_[b]).all()
        tab = np.full((P, S_), pad, np.int32)
        tab[pp, CS_[b] + rank] = vv
        return tab

    streamA_tabs, streamB_tabs, slot_tabs, cnt_tabs = [], [], [], []
    for c in range(NC):
        own = (dst_n >= c * NS) & (dst_n < (c + 1) * NS)
        eA = own & srcA
        eB = own & ~srcA
        sa = fill_stream(dst_n[eA] - c * NS, src_n[eA], KA, CSA, SA, cfg.PAD_A)
        sb = fill_stream(dst_n[eB] - c * NS, src_n[eB] - SPLIT, KB, CSB, SB,
                         cfg.PAD_B)
        streamA_tabs.append(_wrap16(sa))
        streamB_tabs.append(_wrap16(sb))
        slot_tabs.append(make_tab(dst_n[own] - c * NS, src_n[own], S, CS, K, NT))
        own_s = (src_n >= c * NS) & (src_n < (c + 1) * NS)
        cnt_tabs.append(make_tab(src_n[own_s] - c * NS, dst_n[own_s], S2, CS2,
                                 K2, NT))

    return dict(new_of_old=new_of_old, KA=KA, KB=KB, CSA=CSA, CSB=CSB,
                SA=SA, SB=SB, K=K, CS=CS, S=S, K2=K2, CS2=CS2, S2=S2,
                streamA_tabs=streamA_tabs, streamB_tabs=streamB_tabs,
                slot_tabs=slot_tabs, cnt_tabs=cnt_tabs)


# ------------------------------------------------------------- bass helpers

def _raw_gather(nc, out_ap, in_ap, idxs_ap, num_idxs, elem_size, elem_step,
                queue_num):
    """Official dma_gather lowering minus the 256B elem_size assert
    (64B/128B elems HW-verified on this runtime). in_ap is [rows, elem_size]
    with row stride elem_step."""
    gp = nc.gpsimd
    assert idxs_ap.dtype == mybir.dt.int16
    assert in_ap.dtype == out_ap.dtype
    assert ap_utils.ap_is_contiguous(out_ap.ap[1:])
    assert ap_utils.ap_is_contiguous(idxs_ap.ap[1:])
    assert in_ap.ap[-1][1] == out_ap.ap[-1][1] == elem_size
    assert out_ap.ap[0][1] * out_ap.ap[1][1] == round_up_to_multiple(num_idxs, 128)
    assert in_ap.ap[0][0] == elem_step
    stride_bytes = elem_step * mybir.dt.size(in_ap.dtype)
    stride_bytes_256 = exact_div(stride_bytes, 256)
    assert stride_bytes_256 < 256
    _in_ap = gp.lower_ap_dma(in_ap, for_custom_bir_dma=True)
    _idxs_ap = gp.lower_ap(idxs_ap)
    _out_ap = gp.lower_ap(out_ap)
    return gp.add_instruction(
        mybir.InstDMAGatherAnt(
            name=gp.bass.get_next_instruction_name(),
            ins=[*_in_ap, _idxs_ap, gp.lower_val_access(gp.to_reg(num_idxs))],
            outs=[_out_ap],
            transpose=False,
            num_idxs=num_idxs,
            elem_size=elem_size,
            stride_bytes_256=stride_bytes_256,
            gen_mode=0,
            single_packet=True,
            queue_num=queue_num,
            sbuf_tokens_per_rank=0,
            sbuf_free_dim_per_rank=0,
            sbuf_free_dim_pad_per_rank=0,
            sbuf_byte_offset=0,
        )
    )


def _count_degrees(nc, pool, tab_sb, CS_, BPC, zr, deg_out):
    S_ = int(CS_[-1])
    ind = pool.tile([P, S_], F32, tag="ind")
    nc.vector.tensor_scalar(
        out=ind[:], in0=tab_sb[:], scalar1=float(zr), scalar2=None,
        op0=mybir.AluOpType.is_lt,
    )
    for b in range(BPC):
        nc.vector.tensor_reduce(
            out=deg_out[:, b : b + 1],
            in_=ind[:, int(CS_[b]) : int(CS_[b + 1])],
            axis=mybir.AxisListType.X,
            op=mybir.AluOpType.add,
        )


def _norm_from_deg(nc, pool, deg, norm, BPC):
    m = pool.tile([P, BPC], F32, tag="nmask")
    safe = pool.tile([P, BPC], F32, tag="nsafe")
    nc.vector.tensor_scalar(
        out=m[:], in0=deg[:], scalar1=0.0, scalar2=None,
        op0=mybir.AluOpType.is_gt,
    )
    nc.vector.tensor_scalar(
        out=safe[:], in0=deg[:], scalar1=1.0, scalar2=None,
        op0=mybir.AluOpType.max,
    )
    nc.vector.reciprocal(out=safe[:], in_=safe[:])
    nc.scalar.sqrt(out=safe[:], in_=safe[:])
    nc.vector.tensor_mul(out=norm[:], in0=safe[:], in1=m[:])


def _tree(nc, region, w, es):
    """In-place pairwise tree-add of w columns of width es inside region."""
    while w > 1:
        h = (w + 1) // 2
        lo = w - h
        nc.vector.tensor_add(
            out=region[:, : lo * es], in0=region[:, : lo * es],
            in1=region[:, h * es : w * es],
        )
        w = h


def _groups(cfg, KA, KB, capcols):
    """Group consecutive blocks so each window's column total <= capcols."""
    out = []
    b = 0
    while b < cfg.BPC:
        e = b + 1
        ta, tb = KA[b], KB[b]
        while e < cfg.BPC and ta + KA[e] <= capcols and tb + KB[e] <= capcols:
            ta += KA[e]
            tb += KB[e]
            e += 1
        out.append((b, e))
        b = e
    return out


# ------------------------------------------------------------- the program

def build_program(cfg, st, has_bias):
    NS, NT, BPC, SPLIT = cfg.NS, cfg.NT, cfg.BPC, cfg.SPLIT
    KA, KB, CSA, CSB = st["KA"], st["KB"], st["CSA"], st["CSB"]
    SA, SB = st["SA"], st["SB"]
    CS, S, CS2, S2 = st["CS"], st["S"], st["CS2"], st["S2"]

    nc = bacc.Bacc("TRN2", target_bir_lowering=False, debug=False,
                   num_devices=NC, num_swdge_queues=NQ)

    z_in = nc.dram_tensor("z_shard", [NS, DIMS[0]], F32, kind="ExternalInput")
    sA_in = nc.dram_tensor("streamA", [128, SA * 8], I16, kind="ExternalInput")
    sB_in = nc.dram_tensor("streamB", [128, SB * 8], I16, kind="ExternalInput")
    slot_in = nc.dram_tensor("slots", [P, S], I32, kind="ExternalInput")
    cnt_in = nc.dram_tensor("cnts", [P, S2], I32, kind="ExternalInput")
    W_ins = [
        nc.dram_tensor(f"W{l+1}", [DIMS[l] + (1 if has_bias else 0), DIMS[l + 1]],
                       F32, kind="ExternalInput")
        for l in range(4)
    ]
    out_ext = nc.dram_tensor("out_shard", [NS, DIMS[4]], F32,
                             kind="ExternalOutput")

    from concourse.masks import make_identity

    qctr = [0]

    def next_q():
        q = qctr[0] % NQ
        qctr[0] += 1
        return q

    def gather_cols(res_tile, tab, es, idx_sb, c0, c1, col_off):
        """Gather stream columns [c0, c1) into res_tile at column offset."""
        cols = c1 - c0
        done = 0
        while done < cols:
            take = min(8, cols - done)
            ni = take * 128
            dst = res_tile[:, (col_off + done) * es : (col_off + done + take) * es]
            _raw_gather(
                nc, dst.rearrange("p (c d) -> p c d", d=es), tab,
                idx_sb[:, (c0 + done) * 8 : (c0 + done + take) * 8],
                ni, es, TW, next_q(),
            )
            done += take

    tables = [
        nc.dram_tensor(f"tab{l}", [NT, TW], BF16, kind="Internal",
                       addr_space="Shared")
        for l in range(4)
    ]
    with tile.TileContext(nc) as tc:
        with tc.tile_pool(name="dram", bufs=1, space="DRAM") as dram:
            bounces = [dram.tile([NS, TW], BF16, name=f"bnc{l}") for l in range(4)]
            with tc.tile_pool(name="res", bufs=1) as res:
                # ---- persistent loads
                sA_sb = res.tile([128, SA * 8], I16, tag="sA")
                nc.sync.dma_start(out=sA_sb[:], in_=sA_in[:, :])
                sB_sb = res.tile([128, SB * 8], I16, tag="sB")
                nc.sync.dma_start(out=sB_sb[:], in_=sB_in[:, :])
                ident = res.tile([P, P], BF16, tag="ident")
                make_identity(nc, ident[:])
                W_sbs = []
                for l in range(4):
                    win = DIMS[l] + (1 if has_bias else 0)
                    wf = res.tile([win, DIMS[l + 1]], F32, tag=f"Wf{l}")
                    nc.sync.dma_start(out=wf[:], in_=W_ins[l][:, :])
                    wb = res.tile([win, DIMS[l + 1]], BF16, tag=f"Wb{l}")
                    nc.vector.tensor_copy(out=wb[:], in_=wf[:])
                    W_sbs.append(wb)

                # ---- degree norms
                norm_dst = res.tile([P, BPC], F32, tag="ndst")
                norm_src = res.tile([P, BPC], F32, tag="nsrc")
                norm_comb = res.tile([P, BPC], F32, tag="ncomb")
                with tc.tile_pool(name="deg", bufs=1) as dp:
                    slot_sb = dp.tile([P, S], I32, tag="slots")
                    nc.sync.dma_start(out=slot_sb[:], in_=slot_in[:, :])
                    deg = dp.tile([P, BPC], F32, tag="deg")
                    _count_degrees(nc, dp, slot_sb, CS, BPC, NT, deg)
                    _norm_from_deg(nc, dp, deg, norm_dst, BPC)
                    cnt_sb = dp.tile([P, S2], I32, tag="cnts")
                    nc.sync.dma_start(out=cnt_sb[:], in_=cnt_in[:, :])
                    deg2 = dp.tile([P, BPC], F32, tag="deg2")
                    _count_degrees(nc, dp, cnt_sb, CS2, BPC, NT, deg2)
                    _norm_from_deg(nc, dp, deg2, norm_src, BPC)
                    nc.vector.tensor_mul(
                        out=norm_comb[:], in0=norm_dst[:], in1=norm_src[:]
                    )

                # ---- h1 = z * norm_src -> bounce0 -> AllGather tab0
                with tc.tile_pool(name="zp", bufs=3) as zp:
                    for b in range(BPC):
                        zt = zp.tile([P, DIMS[0]], F32, tag="z")
                        nc.sync.dma_start(
                            out=zt[:], in_=z_in[b * P : (b + 1) * P, :]
                        )
                        zb = zp.tile([P, DIMS[0]], BF16, tag="zb")
                        nc.vector.tensor_mul(
                            out=zb[:], in0=zt[:],
                            in1=norm_src[:, b : b + 1].to_broadcast([P, DIMS[0]]),
                        )
                        nc.sync.dma_start(
                            out=bounces[0][b * P : (b + 1) * P, 0 : DIMS[0]],
                            in_=zb[:],
                        )
                nc.gpsimd.collective_compute(
                    "AllGather", mybir.AluOpType.bypass,
                    replica_groups=[list(range(NC))],
                    ins=[bounces[0].opt()], outs=[tables[0][:, :]],
                )

                # ---- layers
                CAP = 64
                groups = _groups(cfg, KA, KB, CAP)
                for l in range(4):
                    es, d_out = DIMS[l], DIMS[l + 1]
                    last = l == 3
                    tabA = tables[l][:, 0:es]
                    tabB = tables[l][SPLIT:, 0:es]
                    with (
                        tc.tile_pool(name=f"g{l}", bufs=2) as gp,
                        tc.tile_pool(name=f"a{l}", bufs=4) as ap,
                        tc.tile_pool(name=f"ps{l}", bufs=4, space="PSUM") as pp,
                    ):
                        for (b0, b1) in groups:
                            a0, a1 = int(CSA[b0]), int(CSA[b1])
                            bb0, bb1 = int(CSB[b0]), int(CSB[b1])
                            gA = gp.tile([P, (a1 - a0) * es], BF16, tag="gA")
                            gB = gp.tile([P, (bb1 - bb0) * es], BF16, tag="gB")
                            gather_cols(gA, tabA, es, sA_sb, a0, a1, 0)
                            gather_cols(gB, tabB, es, sB_sb, bb0, bb1, 0)
                            for b in range(b0, b1):
                                ka, kb = int(KA[b]), int(KB[b])
                                oa = (int(CSA[b]) - a0) * es
                                ob = (int(CSB[b]) - bb0) * es
                                rA = gA[:, oa : oa + ka * es]
                                rB = gB[:, ob : ob + kb * es]
                                _tree(nc, rA, ka, es)
                                _tree(nc, rB, kb, es)
                                acc = ap.tile([P, es], BF16, tag="acc")
                                nc.vector.tensor_add(
                                    out=acc[:], in0=rA[:, :es], in1=rB[:, :es]
                                )
                                if has_bias:
                                    nc.vector.tensor_mul(
                                        out=acc[:], in0=acc[:],
                                        in1=norm_dst[:, b : b + 1]
                                        .to_broadcast([P, es]),
                                    )
                                p1 = pp.tile([es, P], BF16, tag="t1", space="PSUM")
                                nc.tensor.transpose(
                                    out=p1[:], in_=acc[:], identity=ident[:]
                                )
                                ein = es + (1 if has_bias else 0)
                                accT = ap.tile([ein, P], BF16, tag="accT")
                                nc.scalar.copy(out=accT[:es, :], in_=p1[:])
                                if has_bias:
                                    nc.vector.memset(accT[es : es + 1, :], 1.0)
                                p2 = pp.tile([P, d_out], F32, tag="mm",
                                             space="PSUM")
                                nc.tensor.matmul(
                                    out=p2[:], lhsT=accT[:], rhs=W_sbs[l][:],
                                    start=True, stop=True,
                                )
                                if last:
                                    yb = ap.tile([P, d_out], F32, tag="ybf")
                                    nc.scalar.activation(
                                        out=yb[:], in_=p2[:],
                                        func=mybir.ActivationFunctionType.Relu,
                                        scale=(1.0 if has_bias
                                               else norm_dst[:, b : b + 1]),
                                    )
                                    nc.sync.dma_start(
                                        out=out_ext[b * P : (b + 1) * P, :],
                                        in_=yb[:],
                                    )
                                else:
                                    yb = ap.tile([P, d_out], BF16, tag="yb")
                                    sc = norm_src if has_bias else norm_comb
                                    nc.scalar.activation(
                                        out=yb[:], in_=p2[:],
                                        func=mybir.ActivationFunctionType.Relu,
                                        scale=sc[:, b : b + 1],
                                    )
                                    nc.sync.dma_start(
                                        out=bounces[l + 1][
                                            b * P : (b + 1) * P, 0:d_out
                                        ],
                                        in_=yb[:],
                                    )
                    if not last:
                        nc.gpsimd.collective_compute(
                            "AllGather", mybir.AluOpType.bypass,
                            replica_groups=[list(range(NC))],
                            ins=[bounces[l + 1].opt()],
                            outs=[tables[l + 1][:, :]],
                        )
    nc.compile()
    return nc


# ------------------------------------------------------------------ driver

_prog_cache = {}
LAST_RESULTS = []


def kernel(z, src, dst, W1, b1, W2, b2, W3, b3, W4, b4, **extra):
    Ws = [np.ascontiguousarray(np.asarray(w, np.float32)) for w in (W1, W2, W3, W4)]
    bs = [np.ascontiguousarray(np.asarray(b, np.float32)) for b in (b1, b2, b3, b4)]
    z = np.ascontiguousarray(np.asarray(z, np.float32))
    has_bias = any(np.any(b != 0) for b in bs)
    cfg = Cfg(z.shape[0])
    st = build_structures(cfg, src, dst)
    key = (z.shape[0], has_bias, st["SA"], st["SB"], st["S"], st["S2"],
           tuple(st["KA"]), tuple(st["KB"]))
    if key not in _prog_cache:
        _prog_cache[key] = build_program(cfg, st, has_bias)
    nc = _prog_cache[key]
    NS = cfg.NS

    z_all = np.zeros((cfg.NT, DIMS[0]), np.float32)
    z_all[st["new_of_old"]] = z

    if has_bias:
        W_full = [np.concatenate([w, b[None, :]], axis=0) for w, b in zip(Ws, bs)]
    else:
        W_full = Ws

    in_maps = [
        {
            "z_shard": z_all[c * NS : (c + 1) * NS],
            "streamA": st["streamA_tabs"][c],
            "streamB": st["streamB_tabs"][c],
            "slots": st["slot_tabs"][c],
            "cnts": st["cnt_tabs"][c],
            **{f"W{l+1}": W_full[l] for l in range(4)},
        }
        for c in range(NC)
    ]
    LAST_RESULTS.clear()
    _r = run_bass_kernel_spmd(nc, in_maps, list(range(NC)))
    LAST_RESULTS.append(_r)
    out_full = np.concatenate([r["out_shard"] for r in _r.results], axis=0)
    return np.ascontiguousarray(out_full[st["new_of_old"]])
